# revision 1
# baseline (speedup 1.0000x reference)
"""Trainium2 Bass kernel for nn_EqvTransformer (dense_transformer).

Sharding: 8 cores = 4 batches x 2 query-halves. Each core computes the full
attention output for its (batch, 512-query slice) over all 1024 keys and all
8 heads, so no cross-core communication is needed (fc_o is row-local).

Layout choice: "transposed attention" - logits are built as l^T[k, q] tiles
(keys on partitions, queries free). Then:
  - exp() evacuation applies the key-mask as a per-partition ACT bias,
  - the softmax denominator falls out of the A.V matmul via a ones-column
    appended to V (no separate reduce),
  - A^T is directly the moving operand of the A.V matmul (no transposes).
No max-subtraction is needed: logits are bounded (~|l|<20) for present keys,
so exp() is safe in fp32; reference softmax(l) == exp(l)/sum(exp(l)).

Presence masking (exactly reproducing the reference):
  - key k absent  -> A[q,k]=0: exp bias -1e30*(1-p_k) per k-partition.
  - query q absent-> reference yields uniform A over ALL keys, i.e.
    Oh[q] = mean(V). We instead zero the whole column (rank-1 -1e30*(1-p_q)
    folded into the pair-MLP matmul via an extra input row), fix the
    denominator (s += 1-p_q), and add (1-p_q)*mean(V) back to O before fc_o.
"""

import sys, os

sys.path.insert(0, "/opt/trn_rl_repo")

import numpy as np
import ml_dtypes

import concourse.bass as bass
import concourse.tile as tile
from concourse import bacc, mybir
from concourse import bass_utils

B, N, D, H = 4, 1024, 512, 8
HD = D // H          # 64
NQ = 512             # queries per core
NKC = N // 128       # 8 key chunks of 128
NDT = D // 128       # 4 dout tiles of 128
BIGNEG = -1.0e30

F32 = mybir.dt.float32
F32R = mybir.dt.float32r
BF16 = mybir.dt.bfloat16
AF = mybir.ActivationFunctionType
OP = mybir.AluOpType
BF16NP = ml_dtypes.bfloat16




def build_program(W1, b1, W2, b2):
    """Build the SPMD program (same for all 8 cores; per-core data differs).

    W1 (H,3,3), b1 (H,3), W2 (H,3), b2 (H,) are baked into instruction
    immediates (the kernel is compiled per call, so the weights are known).
    """
    W1 = np.asarray(W1, np.float64)
    b1 = np.asarray(b1, np.float64)
    W2 = np.asarray(W2, np.float64)

    nc = bacc.Bacc("TRN2", target_bir_lowering=False, debug=False, num_devices=8)

    dram = {}

    def din(name, shape, dtype=F32):
        dram[name] = nc.dram_tensor(name, shape, dtype, kind="ExternalInput").ap()
        return dram[name]

    t_ytq = din("ytq", [D, NQ], F32R)        # presence-scaled Y^T slice (Q rhs)
    t_ytqr = din("ytqr", [D, NQ], F32R)      # raw Y^T slice (Vt_q rhs)
    t_yt = din("yt", [D, N], F32R)           # raw Y^T full (K rhs, V lhsT)
    t_wqt = din("wqt", [D, D], F32R)         # Wq.T / sqrt(D)
    t_wkt = din("wkt", [D, D], F32R)
    t_wvt = din("wvt", [D, D], F32R)
    t_wot = din("wot", [D, D], F32R)
    t_xt = din("xt", [3, N, NQ], BF16)  # X_pairs channel planes, [c, k, q]
    t_maskq = din("maskq", [1, NQ], F32R)     # -BIG*(1-p_q) rank-1 row
    t_expb = din("expb", [H, N])        # -BIG*(1-p_k) + b2[h]
    t_ompq = din("ompq", [1, NQ])       # 1 - p_q
    t_bq = din("bq", [D])
    t_bk = din("bk", [D])
    t_bv = din("bv", [D])
    t_bo = din("bo", [D])
    t_bvrow = din("bvrow", [1, D], F32R)      # bv as a row (V-natural bias aug)
    t_mv = din("mv", [D])               # mean(V) over all tokens (host)
    t_ones = din("ones", [1, 128], F32R)
    t_out = nc.dram_tensor("out_t", [D, NQ], F32, kind="ExternalOutput").ap()

    with tile.TileContext(nc) as tc:
        with (
            tc.tile_pool(name="const", bufs=1) as const,
            tc.tile_pool(name="work", bufs=3) as work,
            tc.tile_pool(name="loc", bufs=3) as locp,
            tc.tile_pool(name="av", bufs=3) as avp,
            tc.tile_pool(name="outp", bufs=2) as outp,
            tc.tile_pool(name="psA", bufs=2, space="PSUM") as psA,
            tc.tile_pool(name="psL", bufs=2, space="PSUM") as psL,
            tc.tile_pool(name="psO", bufs=2, space="PSUM") as psO,
            tc.tile_pool(name="dram", bufs=1, space="DRAM") as dramp,
        ):
            # ---------------- Phase 0: resident loads ----------------
            yt_sb = const.tile([128, NDT, N], F32R)
            nc.sync.dma_start(yt_sb, t_yt.rearrange("(dt p) n -> p dt n", p=128))
            ytq_sb = const.tile([128, NDT, NQ], F32R)
            nc.sync.dma_start(ytq_sb, t_ytq.rearrange("(dt p) n -> p dt n", p=128))
            ytqr_sb = const.tile([128, NDT, NQ], F32R)
            nc.sync.dma_start(ytqr_sb, t_ytqr.rearrange("(dt p) n -> p dt n", p=128))
            w_sb = {}
            for nm, t in (("q", t_wqt), ("k", t_wkt), ("v", t_wvt), ("o", t_wot)):
                w_sb[nm] = const.tile([128, NDT, D], F32R, tag=f"w{nm}", name=f"w{nm}_sb")
                nc.sync.dma_start(w_sb[nm], t.rearrange("(kt p) d -> p kt d", p=128))
            xt_sb = const.tile([128, NKC, 3, NQ], BF16)
            for c in range(3):
                nc.sync.dma_start(
                    xt_sb[:, :, c, :],
                    t_xt[c].rearrange("(kc p) q -> p kc q", p=128),
                )
            maskq_sb = const.tile([1, NQ], F32R)
            nc.sync.dma_start(maskq_sb, t_maskq)
            expb_sb = const.tile([128, H, NKC], F32)
            nc.sync.dma_start(expb_sb, t_expb.rearrange("h (kc p) -> p h kc", p=128))
            ompq_sb = const.tile([1, NQ], F32)
            nc.sync.dma_start(ompq_sb, t_ompq)
            ompq_bc = const.tile([128, NQ], F32)
            src = t_ompq[0:1, :]
            nc.sync.dma_start(
                ompq_bc,
                bass.AP(tensor=src.tensor, offset=src.offset, ap=[[0, 128], [1, NQ]]),
            )
            bias_sb = {}
            for nm, t in (("q", t_bq), ("k", t_bk), ("v", t_bv), ("o", t_bo), ("mv", t_mv)):
                bias_sb[nm] = const.tile([128, NDT], F32, tag=f"b{nm}", name=f"b{nm}_sb")
                nc.sync.dma_start(bias_sb[nm], t.rearrange("(dt p) -> p dt", p=128))
            bvrow_sb = const.tile([1, D], F32R)
            nc.sync.dma_start(bvrow_sb, t_bvrow)
            ones_row = const.tile([1, 128], F32R)
            nc.sync.dma_start(ones_row, t_ones)

            # ---------------- Phase 1: projections ----------------
            # Q^T (presence-scaled rhs), K^T: [dout-part, token-free]
            qt_sb = const.tile([128, NDT, NQ], F32R)
            kt_sb = const.tile([128, NDT, N], F32R)
            for dt in range(NDT):
                ps = psA.tile([128, NQ], F32, tag="proj")
                for kt in range(NDT):
                    nc.tensor.matmul(
                        ps,
                        (w_sb["q"][:, kt, dt * 128:(dt + 1) * 128]),
                        (ytq_sb[:, kt, :]),
                        start=(kt == 0), stop=(kt == NDT - 1),
                    )
                nc.scalar.activation(
                    qt_sb[:, dt, :], ps, AF.Identity, bias=bias_sb["q"][:, dt:dt + 1]
                )
                for th in range(2):
                    ps2 = psA.tile([128, NQ], F32, tag="proj")
                    for kt in range(NDT):
                        nc.tensor.matmul(
                            ps2,
                            (w_sb["k"][:, kt, dt * 128:(dt + 1) * 128]),
                            (yt_sb[:, kt, th * NQ:(th + 1) * NQ]),
                            start=(kt == 0), stop=(kt == NDT - 1),
                        )
                    nc.scalar.activation(
                        kt_sb[:, dt, th * NQ:(th + 1) * NQ], ps2, AF.Identity,
                        bias=bias_sb["k"][:, dt:dt + 1],
                    )
            # V natural [token-part, dout-free] (bf16, with ones column per head)
            v_sb = const.tile([128, NKC, H, HD + 1], BF16)
            nc.vector.memset(v_sb[:, :, :, HD:HD + 1], 1.0)
            for tt in range(NKC):
                ps = psA.tile([128, D], F32, tag="proj")
                for kt in range(NDT):
                    nc.tensor.matmul(
                        ps,
                        (yt_sb[:, kt, tt * 128:(tt + 1) * 128]),
                        (w_sb["v"][:, kt, :]),
                        start=(kt == 0), stop=False,
                    )
                nc.tensor.matmul(
                    ps, (ones_row), (bvrow_sb), start=False, stop=True
                )
                nc.vector.tensor_copy(
                    v_sb[:, tt, :, 0:HD], ps.rearrange("p (h d) -> p h d", h=H)
                )
            # V^T for the query slice [dout-part, q-free] (residual + fc_o input)
            vtq_sb = const.tile([128, NDT, NQ], F32R)
            for dt in range(NDT):
                ps = psA.tile([128, NQ], F32, tag="proj")
                for kt in range(NDT):
                    nc.tensor.matmul(
                        ps,
                        (w_sb["v"][:, kt, dt * 128:(dt + 1) * 128]),
                        (ytqr_sb[:, kt, :]),
                        start=(kt == 0), stop=(kt == NDT - 1),
                    )
                nc.scalar.activation(
                    vtq_sb[:, dt, :], ps, AF.Identity, bias=bias_sb["v"][:, dt:dt + 1]
                )

            # ---------------- Phase 2: attention ----------------
            oht_sb = const.tile([128, NDT, NQ], F32)
            r_tiles = []
            for h in range(H):
                po = psO.tile([HD + 1, NQ], F32, tag="po")
                for kc in range(NKC):
                    ps = psL.tile([128, NQ], F32, tag="l")
                    # content logits^T + rank-1 query mask
                    nc.tensor.matmul(
                        ps,
                        (kt_sb[64 * (h % 2):64 * (h % 2) + 64, h // 2,
                                  kc * 128:(kc + 1) * 128]),
                        (qt_sb[64 * (h % 2):64 * (h % 2) + 64, h // 2, :]),
                        start=True, stop=False,
                    )
                    nc.tensor.matmul(
                        ps, (ones_row), (maskq_sb), start=False, stop=True
                    )
                    # pairwise MLP: loc = sum_o W2[h,o]*relu(<W1[h,o],x>+b1[h,o])
                    lacc = locp.tile([128, NQ], BF16, tag="lacc")
                    for o in range(3):
                        w0, w1, w2 = (float(W1[h, o, c]) for c in range(3))
                        z = locp.tile([128, NQ], BF16, tag="z")
                        nc.vector.tensor_scalar(
                            z, xt_sb[:, kc, 0, :], w0, float(b1[h, o]),
                            OP.mult, OP.add,
                        )
                        nc.vector.scalar_tensor_tensor(
                            z, xt_sb[:, kc, 1, :], w1, z, OP.mult, OP.add
                        )
                        nc.vector.scalar_tensor_tensor(
                            z, xt_sb[:, kc, 2, :], w2, z, OP.mult, OP.add
                        )
                        if o == 0:
                            nc.vector.tensor_scalar(
                                lacc, z, 0.0, float(W2[h, o]), OP.max, OP.mult
                            )
                        else:
                            t = locp.tile([128, NQ], BF16, tag="t")
                            nc.vector.tensor_scalar(
                                t, z, 0.0, float(W2[h, o]), OP.max, OP.mult
                            )
                            nc.vector.tensor_add(lacc, lacc, t)
                    nc.vector.scalar_tensor_tensor(
                        ps, lacc, 1.0, ps, OP.mult, OP.add
                    )
                    # A^T = exp(l^T + key-mask-bias + b2)
                    a = avp.tile([128, NQ], BF16, tag="a")
                    nc.scalar.activation(
                        a, ps, AF.Exp, bias=expb_sb[:, h, kc:kc + 1]
                    )
                    # Oh^T[h] += V[kc,h-cols|ones]^T . A^T
                    nc.tensor.matmul(
                        po, v_sb[:, kc, h, :], a,
                        start=(kc == 0), stop=(kc == NKC - 1),
                    )
                # denominator fix + reciprocal; evacuate unnormalized Oh^T
                s_sb = work.tile([1, NQ], F32, tag="s", bufs=2)
                nc.vector.scalar_tensor_tensor(
                    s_sb, po[HD:HD + 1, :], 1.0, ompq_sb, OP.mult, OP.add
                )
                rt = work.tile([1, NQ], F32, tag=f"r{h}", name=f"rrow{h}", bufs=1)
                nc.vector.reciprocal(rt, s_sb)
                r_tiles.append(rt)
                nc.vector.tensor_copy(
                    oht_sb[64 * (h % 2):64 * (h % 2) + 64, h // 2, :], po[0:HD, :]
                )

            # broadcast per-head reciprocals across partitions via DRAM bounce
            rb_dram = dramp.tile([H, NQ], F32)
            for h in range(H):
                nc.sync.dma_start(rb_dram[h:h + 1, :], r_tiles[h])
            rb_sb = const.tile([128, NDT, NQ], F32)
            for dt in range(NDT):
                for hh in range(2):
                    src = rb_dram[2 * dt + hh:2 * dt + hh + 1, :]
                    nc.sync.dma_start(
                        rb_sb[64 * hh:64 * hh + 64, dt, :],
                        bass.AP(tensor=src.tensor, offset=src.offset,
                                ap=[[0, 64], [1, NQ]]),
                    )

            # ---------------- Phase 3: residual + fc_o ----------------
            opre_sb = const.tile([128, NDT, NQ], F32R)
            for dt in range(NDT):
                # OPre = Vq + r*Oh + (1-p_q)*meanV
                nc.vector.scalar_tensor_tensor(
                    opre_sb[:, dt, :], oht_sb[:, dt, :], 1.0, rb_sb[:, dt, :],
                    OP.mult, OP.mult,
                )
                nc.vector.tensor_add(
                    opre_sb[:, dt, :], opre_sb[:, dt, :], vtq_sb[:, dt, :]
                )
                nc.vector.scalar_tensor_tensor(
                    opre_sb[:, dt, :], ompq_bc, bias_sb["mv"][:, dt:dt + 1],
                    opre_sb[:, dt, :], OP.mult, OP.add,
                )
            for dt in range(NDT):
                ps = psA.tile([128, NQ], F32, tag="proj")
                for kt in range(NDT):
                    nc.tensor.matmul(
                        ps,
                        (w_sb["o"][:, kt, dt * 128:(dt + 1) * 128]),
                        (opre_sb[:, kt, :]),
                        start=(kt == 0), stop=(kt == NDT - 1),
                    )
                relu_sb = outp.tile([128, NQ], F32, tag="relu")
                nc.scalar.activation(
                    relu_sb, ps, AF.Relu, bias=bias_sb["o"][:, dt:dt + 1]
                )
                of_sb = outp.tile([128, NQ], F32, tag="of")
                nc.vector.tensor_add(of_sb, relu_sb, opre_sb[:, dt, :])
                nc.sync.dma_start(t_out[dt * 128:(dt + 1) * 128, :], of_sb)

    nc.compile()
    return nc


def make_in_maps(inputs):
    """Host-side prep: returns the per-core input dicts."""
    Y = np.asarray(inputs["Y_lift"], np.float32)
    X = np.asarray(inputs["X_pairs"], np.float32)
    pres = np.asarray(inputs["presence"], np.float32)
    Wq = np.asarray(inputs["Wq"], np.float32)
    Wk = np.asarray(inputs["Wk"], np.float32)
    Wv = np.asarray(inputs["Wv"], np.float32)
    Wo = np.asarray(inputs["Wo"], np.float32)
    bq = np.asarray(inputs["bq"], np.float32)
    bk = np.asarray(inputs["bk"], np.float32)
    bv = np.asarray(inputs["bv"], np.float32)
    bo = np.asarray(inputs["bo"], np.float32)
    b2 = np.asarray(inputs["b2"], np.float32)

    inv_sqrt = np.float32(1.0 / np.sqrt(D))
    WqT = np.ascontiguousarray(Wq.T * inv_sqrt)
    WkT = np.ascontiguousarray(Wk.T)
    WvT = np.ascontiguousarray(Wv.T)
    WoT = np.ascontiguousarray(Wo.T)

    Yt = np.ascontiguousarray(Y.transpose(0, 2, 1))            # (B, D, N)
    YtQ = Yt * pres[:, None, :]                                 # presence-scaled
    XT = np.ascontiguousarray(X.transpose(0, 3, 2, 1))          # (B, 3, k, q)
    V_full = Y @ Wv.T + bv                                      # (B, N, D) host
    meanV = V_full.mean(axis=1).astype(np.float32)              # (B, D)

    in_maps = []
    for c in range(8):
        b, qh = c // 2, c % 2
        qsl = slice(qh * NQ, (qh + 1) * NQ)
        pkb = (BIGNEG * (1.0 - pres[b])).astype(np.float32)     # (N,)
        expb = (pkb[None, :] + b2[:, None]).astype(np.float32)  # (H, N)
        in_maps.append({
            "ytq": np.ascontiguousarray(YtQ[b][:, qsl]),
            "ytqr": np.ascontiguousarray(Yt[b][:, qsl]),
            "yt": Yt[b],
            "wqt": WqT, "wkt": WkT, "wvt": WvT, "wot": WoT,
            "xt": np.ascontiguousarray(XT[b][:, :, qsl]).astype(BF16NP),
            "maskq": np.ascontiguousarray(
                BIGNEG * (1.0 - pres[b, qsl])).astype(np.float32).reshape(1, NQ),
            "expb": expb,
            "ompq": (1.0 - pres[b, qsl]).astype(np.float32).reshape(1, NQ),
            "bq": bq, "bk": bk, "bv": bv, "bo": bo,
            "bvrow": bv.reshape(1, D),
            "ones": np.ones((1, 128), np.float32),
            "mv": meanV[b],
        })
    return in_maps


def assemble_output(results):
    out = np.empty((B, N, D), np.float32)
    for c in range(8):
        b, qh = c // 2, c % 2
        out[b, qh * NQ:(qh + 1) * NQ, :] = results[c]["out_t"].T
    return out


def kernel(**inputs):
    nc = build_program(inputs["W1"], inputs["b1"], inputs["W2"], inputs["b2"])
    in_maps = make_in_maps(inputs)
    trace = bool(int(os.environ.get("KERNEL_TRACE", "0")))
    res = bass_utils.run_bass_kernel_spmd(
        nc, in_maps, core_ids=list(range(8)), trace=trace
    )
    kernel.last_result = res
    return assemble_output(res.results)



# revision 11
# speedup vs baseline: 1.7031x; 1.7031x over previous
"""Trainium2 Bass kernel for nn_EqvTransformer (dense_transformer).

Sharding: 8 cores = 4 batches x 2 query-halves. Each core computes the full
attention output for its (batch, 512-query slice) over all 1024 keys and all
8 heads; no cross-core communication (fc_o is row-local).

Layout: "transposed attention" - logits are built as l^T[k, q] tiles (keys on
partitions, queries free):
  - content logits via a 65-row contract: rows 0-63 are K^T/Q^T for the head,
    row 64 is (ones | -1e30*(1-p_q)) so the absent-query mask rides the same
    matmul for free,
  - the pairwise-MLP location logits are accumulated into the same PSUM tile
    by identity-stationary matmuls (PE adds tiles for free),
  - exp() evacuation applies the key-mask + b2[h] as a per-partition ACT bias,
  - the softmax denominator falls out of the A.V matmul via a ones-column
    appended to V.

Pairwise MLP (the arithmetic bottleneck) is restructured for DVE perf modes:
per hidden unit, normalize by the largest |W1| coefficient p (ratios <= 1 so
bf16 stays accurate):
    t1 = x_c1 * r1 + b'      (tensor_scalar, 4x mode)
    t2 = x_c2 * r2           (tensor_scalar, 4x mode)
    z  = x_p + t1; z += t2   (tensor_tensor, 2x mode)
    r  = max/min(z, 0) * (a*W2)  (tensor_scalar, 4x mode; sign of a folds in)
then PSUM += I.r on the PE. scalar_tensor_tensor (no DVE perf modes) is
avoided entirely in the hot loop.

Absent queries (p_q=0): reference yields uniform A over ALL keys, i.e.
Oh[q] = mean(V). The -1e30 row zeroes the column, the denominator is fixed
(s += 1-p_q), and (1-p_q)*mean(V) is added back before fc_o.
"""

import sys, os

sys.path.insert(0, "/opt/trn_rl_repo")

import numpy as np
import ml_dtypes

import concourse.bass as bass
import concourse.tile as tile
from concourse import bacc, mybir
from concourse import bass_utils

B, N, D, H = 4, 1024, 512, 8
HD = D // H          # 64
NQ = 512             # queries per core
NKC = N // 128       # 8 key chunks of 128
NDT = D // 128       # 4 dout tiles of 128
KHALF = 2            # key halves for the MLP tiling
KCH = NKC // KHALF   # 4 key chunks per half
BIGNEG = -1.0e30

F32 = mybir.dt.float32
F32R = mybir.dt.float32r
BF16 = mybir.dt.bfloat16
AF = mybir.ActivationFunctionType
OP = mybir.AluOpType
BF16NP = ml_dtypes.bfloat16

# Units (h, o) routed through PE/ACT (z assembled in PSUM by identity
# matmuls, relu-evacuated by the scalar engine) instead of pure DVE.
N_ROUTE_B = int(os.environ.get("KERNEL_NB", "0"))


def _unit_params(W1, b1, W2):
    """Per (h, o): pivot channel, ratios, scales for the normalized MLP."""
    units = []
    for h in range(H):
        for o in range(3):
            w = [float(W1[h, o, c]) for c in range(3)]
            p = int(np.argmax(np.abs(w)))
            a = w[p]
            if a == 0.0:
                a = 1e-30
            c1, c2 = [c for c in range(3) if c != p]
            units.append({
                "h": h, "o": o, "p": p, "c1": c1, "c2": c2,
                "r1": w[c1] / a, "r2": w[c2] / a, "bn": float(b1[h, o]) / a,
                "a": a, "w2": float(W2[h, o]),
            })
    return units


def build_program(W1, b1, W2, b2):
    """Build the SPMD program (same for all 8 cores; per-core data differs).

    W1 (H,3,3), b1 (H,3), W2 (H,3), b2 (H,) are baked into instruction
    immediates (the kernel is compiled per call, so the weights are known).
    """
    W1 = np.asarray(W1, np.float64)
    b1 = np.asarray(b1, np.float64)
    W2 = np.asarray(W2, np.float64)
    units = _unit_params(W1, b1, W2)
    # Route-B set: units with the largest |a*w2| get PE/ACT treatment first
    # (arbitrary but deterministic); tuned via N_ROUTE_B.
    route_b = set()
    if N_ROUTE_B:
        order = sorted(range(24), key=lambda i: -abs(units[i]["a"] * units[i]["w2"]))
        route_b = set(order[:N_ROUTE_B])

    nc = bacc.Bacc("TRN2", target_bir_lowering=False, debug=False, num_devices=8)

    dram = {}

    def din(name, shape, dtype=F32):
        dram[name] = nc.dram_tensor(name, shape, dtype, kind="ExternalInput").ap()
        return dram[name]

    t_yt = din("yt", [D, N], F32R)           # raw Y^T full (proj rhs)
    t_wqt = din("wqt", [D, D], F32R)         # Wq.T / sqrt(D)
    t_wkt = din("wkt", [D, D], F32R)
    t_wvt = din("wvt", [D, D], F32R)
    t_wot = din("wot", [D, D], F32R)
    t_xt = din("xt", [3, N, NQ], BF16)       # X_pairs channel planes, [c, k, q]
    t_maskq = din("maskq", [1, NQ], BF16)    # -BIG*(1-p_q) row (bf16)
    t_expb = din("expb", [H, N])             # -BIG*(1-p_k) + b2[h]
    t_ompq = din("ompq", [1, NQ])            # 1 - p_q
    t_bq = din("bq", [D])
    t_bk = din("bk", [D])
    t_bv = din("bv", [D])
    t_bo = din("bo", [D])
    t_bvrow = din("bvrow", [1, D], F32R)     # bv as a row (V-natural bias aug)
    t_mv = din("mv", [D])                    # mean(V) over all tokens (host)
    t_ones = din("ones", [1, 128], F32R)
    t_ident = din("ident", [128, 128], BF16)  # identity (acc stationary)
    t_out = nc.dram_tensor("out_t", [D, NQ], F32, kind="ExternalOutput").ap()

    qsl = slice(0, NQ)  # query slice within yt is host-prepared (yt IS the
    # full batch Y^T; the q-slice columns are passed via a separate view)
    t_ytq = din("ytq", [D, NQ], F32R)        # raw Y^T q-slice (Q/Vq rhs)

    with tile.TileContext(nc) as tc:
        with (
            tc.tile_pool(name="const", bufs=1) as const,
            tc.tile_pool(name="work", bufs=2) as work,
            tc.tile_pool(name="mlpt", bufs=2) as mlpt,
            tc.tile_pool(name="mlpr", bufs=2) as mlpr,
            tc.tile_pool(name="av", bufs=3) as avp,
            tc.tile_pool(name="outp", bufs=1) as outp,
            tc.tile_pool(name="psA", bufs=2, space="PSUM") as psA,
            tc.tile_pool(name="psL", bufs=4, space="PSUM") as psL,
            tc.tile_pool(name="psO", bufs=2, space="PSUM") as psO,
            tc.tile_pool(name="dram", bufs=1, space="DRAM") as dramp,
        ):
            # ---------------- Phase 0: resident loads ----------------
            yt_sb = const.tile([128, NDT, N], F32R)
            nc.sync.dma_start(yt_sb, t_yt.rearrange("(dt p) n -> p dt n", p=128))
            ytq_sb = const.tile([128, NDT, NQ], F32R)
            nc.sync.dma_start(ytq_sb, t_ytq.rearrange("(dt p) n -> p dt n", p=128))
            w_sb = {}
            for nm, t in (("q", t_wqt), ("k", t_wkt), ("v", t_wvt), ("o", t_wot)):
                w_sb[nm] = const.tile([128, NDT, D], F32R, tag=f"w{nm}", name=f"w{nm}_sb")
                nc.sync.dma_start(w_sb[nm], t.rearrange("(kt p) d -> p kt d", p=128))
            xt_sb = const.tile([128, NKC, 3, NQ], BF16)
            for c in range(3):
                nc.sync.dma_start(
                    xt_sb[:, :, c, :],
                    t_xt[c].rearrange("(kc p) q -> p kc q", p=128),
                )
            expb_sb = const.tile([128, H, NKC], F32)
            nc.sync.dma_start(expb_sb, t_expb.rearrange("h (kc p) -> p h kc", p=128))
            ompq_sb = const.tile([1, NQ], F32)
            nc.sync.dma_start(ompq_sb, t_ompq)
            src = t_ompq[0:1, :]
            ompq_bc = const.tile([128, NQ], F32)
            nc.sync.dma_start(
                ompq_bc,
                bass.AP(tensor=src.tensor, offset=src.offset, ap=[[0, 128], [1, NQ]]),
            )
            bias_sb = {}
            for nm, t in (("q", t_bq), ("k", t_bk), ("v", t_bv), ("o", t_bo), ("mv", t_mv)):
                bias_sb[nm] = const.tile([128, NDT], F32, tag=f"b{nm}", name=f"b{nm}_sb")
                nc.sync.dma_start(bias_sb[nm], t.rearrange("(dt p) -> p dt", p=128))
            bvrow_sb = const.tile([1, D], F32R)
            nc.sync.dma_start(bvrow_sb, t_bvrow)
            ones_row = const.tile([1, 128], F32R)
            nc.sync.dma_start(ones_row, t_ones)
            ident_sb = const.tile([128, 128], BF16)
            nc.sync.dma_start(ident_sb, t_ident)
            # Scaled identities for route-B units (W2 applied by the PE).
            w2i_sb = {}
            for i in sorted(route_b):
                u = units[i]
                w2i_sb[i] = const.tile([128, 128], BF16, tag=f"w2i{i}", name=f"w2i{i}")
                nc.vector.tensor_scalar(
                    w2i_sb[i], ident_sb, float(u["w2"]), 0.0, OP.mult, OP.add
                )

            # ---------------- Phase 1: projections ----------------
            # qt_h: [65, NQ] bf16 per head (rows 0-63 Q^T/sqrt(D), row 64 maskq)
            # kt_h: [65, N]  bf16 per head (rows 0-63 K^T, row 64 ones)
            qt_h = [const.tile([65, NQ], BF16, tag=f"qt{h}", name=f"qt{h}") for h in range(H)]
            kt_h = [const.tile([65, N], BF16, tag=f"kt{h}", name=f"kt{h}") for h in range(H)]
            for h in range(H):
                nc.sync.dma_start(qt_h[h][64:65, :], t_maskq)
                nc.vector.memset(kt_h[h][64:65, :], 1.0)
            for dt in range(NDT):
                ps = psA.tile([128, NQ], F32, tag="proj")
                for kt in range(NDT):
                    nc.tensor.matmul(
                        ps,
                        (w_sb["q"][:, kt, dt * 128:(dt + 1) * 128]),
                        (ytq_sb[:, kt, :]),
                        start=(kt == 0), stop=(kt == NDT - 1),
                    )
                for hh in range(2):
                    nc.scalar.activation(
                        qt_h[2 * dt + hh][0:64, :], ps[64 * hh:64 * hh + 64, :],
                        AF.Identity, bias=bias_sb["q"][64 * hh:64 * hh + 64, dt:dt + 1],
                    )
                for th in range(2):
                    ps2 = psA.tile([128, NQ], F32, tag="proj")
                    for kt in range(NDT):
                        nc.tensor.matmul(
                            ps2,
                            (w_sb["k"][:, kt, dt * 128:(dt + 1) * 128]),
                            (yt_sb[:, kt, th * NQ:(th + 1) * NQ]),
                            start=(kt == 0), stop=(kt == NDT - 1),
                        )
                    for hh in range(2):
                        nc.scalar.activation(
                            kt_h[2 * dt + hh][0:64, th * NQ:(th + 1) * NQ],
                            ps2[64 * hh:64 * hh + 64, :],
                            AF.Identity,
                            bias=bias_sb["k"][64 * hh:64 * hh + 64, dt:dt + 1],
                        )
            # V natural [token-part, dout-free] (bf16, with ones column per head)
            v_sb = const.tile([128, NKC, H, HD + 1], BF16)
            nc.vector.memset(v_sb[:, :, :, HD:HD + 1], 1.0)
            for tt in range(NKC):
                ps = psA.tile([128, D], F32, tag="proj")
                for kt in range(NDT):
                    nc.tensor.matmul(
                        ps,
                        (yt_sb[:, kt, tt * 128:(tt + 1) * 128]),
                        (w_sb["v"][:, kt, :]),
                        start=(kt == 0), stop=False,
                    )
                nc.tensor.matmul(
                    ps, (ones_row), (bvrow_sb), start=False, stop=True
                )
                nc.scalar.activation(
                    v_sb[:, tt, :, 0:HD], ps.rearrange("p (h d) -> p h d", h=H),
                    AF.Identity,
                )
            # V^T for the query slice [dout-part, q-free] (residual + fc_o input)
            vtq_sb = const.tile([128, NDT, NQ], BF16)
            for dt in range(NDT):
                ps = psA.tile([128, NQ], F32, tag="proj")
                for kt in range(NDT):
                    nc.tensor.matmul(
                        ps,
                        (w_sb["v"][:, kt, dt * 128:(dt + 1) * 128]),
                        (ytq_sb[:, kt, :]),
                        start=(kt == 0), stop=(kt == NDT - 1),
                    )
                nc.scalar.activation(
                    vtq_sb[:, dt, :], ps, AF.Identity, bias=bias_sb["v"][:, dt:dt + 1]
                )

            # ---------------- Phase 2: attention ----------------
            oht_sb = const.tile([128, NDT, NQ], F32R)
            rden_tiles = []
            for h in range(H):
                po = psO.tile([HD + 1, NQ], F32, tag="po")
                for kh in range(KHALF):
                    # --- pairwise MLP for this (head, key-half) ---
                    r_tiles = []
                    psz_tiles = {}
                    for o in range(3):
                        u = units[3 * h + o]
                        xp = xt_sb[:, kh * KCH:(kh + 1) * KCH, u["p"], :]
                        x1 = xt_sb[:, kh * KCH:(kh + 1) * KCH, u["c1"], :]
                        x2 = xt_sb[:, kh * KCH:(kh + 1) * KCH, u["c2"], :]
                        t1 = mlpt.tile([128, KCH, NQ], BF16, tag="t1")
                        t2 = mlpt.tile([128, KCH, NQ], BF16, tag="t2")
                        nc.vector.tensor_scalar(
                            t1, x1, u["r1"], u["bn"], OP.mult, OP.add
                        )
                        nc.vector.tensor_scalar(
                            t2, x2, u["r2"], 0.0, OP.mult, OP.add
                        )
                        if (3 * h + o) not in route_b:
                            nc.vector.tensor_add(t1, t1, xp)
                            nc.vector.tensor_add(t1, t1, t2)
                            r = mlpr.tile([128, KCH, NQ], BF16, tag=f"r{o}")
                            nc.vector.tensor_scalar(
                                r, t1, 0.0, u["a"] * u["w2"],
                                OP.max if u["a"] > 0 else OP.min, OP.mult,
                            )
                            r_tiles.append((r, ident_sb))
                        else:
                            # route B: PE assembles z in PSUM, ACT relu-evacuates
                            r = mlpr.tile([128, KCH, NQ], BF16, tag=f"r{o}")
                            for j in range(KCH):
                                psz = psL.tile([128, NQ], F32, tag="z")
                                nc.tensor.matmul(psz, ident_sb, xp[:, j, :],
                                                 start=True, stop=False)
                                nc.tensor.matmul(psz, ident_sb, t1[:, j, :],
                                                 start=False, stop=False)
                                nc.tensor.matmul(psz, ident_sb, t2[:, j, :],
                                                 start=False, stop=True)
                                nc.scalar.activation(
                                    r[:, j, :], psz, AF.Relu, scale=float(u["a"])
                                )
                            r_tiles.append((r, w2i_sb[3 * h + o]))
                    # --- content logits + MLP accumulation + exp + A.V ---
                    ps_tiles = []
                    for j in range(KCH):
                        kc = kh * KCH + j
                        ps = psL.tile([128, NQ], F32, tag="l")
                        nc.tensor.matmul(
                            ps,
                            (kt_h[h][:, kc * 128:(kc + 1) * 128]),
                            (qt_h[h]),
                            start=True, stop=False,
                        )
                        ps_tiles.append(ps)
                    for j in range(KCH):
                        ps = ps_tiles[j]
                        for o in range(3):
                            r, stat = r_tiles[o]
                            nc.tensor.matmul(
                                ps, stat, r[:, j, :],
                                start=False, stop=(o == 2),
                            )
                    for j in range(KCH):
                        kc = kh * KCH + j
                        a = avp.tile([128, NQ], BF16, tag="a")
                        nc.scalar.activation(
                            a, ps_tiles[j], AF.Exp, bias=expb_sb[:, h, kc:kc + 1]
                        )
                        nc.tensor.matmul(
                            po, v_sb[:, kc, h, :], a,
                            start=(kc == 0), stop=(kc == NKC - 1),
                        )
                # denominator row (+ (1-p_q) fix, approx reciprocal) + Oh^T evac
                s_sb = work.tile([1, NQ], F32, tag="s", bufs=2)
                nc.vector.tensor_add(s_sb, po[HD:HD + 1, :], ompq_sb)
                rt = work.tile([1, NQ], F32, tag=f"r{h}", name=f"rrow{h}", bufs=1)
                nc.vector.reciprocal_approx_fast(rt, s_sb)
                rden_tiles.append(rt)
                nc.scalar.activation(
                    oht_sb[64 * (h % 2):64 * (h % 2) + 64, h // 2, :], po[0:HD, :],
                    AF.Identity,
                )

            # broadcast per-head reciprocals across partitions via DRAM bounce
            rb_dram = dramp.tile([H, NQ], F32)
            for h in range(H):
                nc.sync.dma_start(rb_dram[h:h + 1, :], rden_tiles[h])
            rb_sb = const.tile([128, NDT, NQ], F32)
            for dt in range(NDT):
                for hh in range(2):
                    src = rb_dram[2 * dt + hh:2 * dt + hh + 1, :]
                    nc.sync.dma_start(
                        rb_sb[64 * hh:64 * hh + 64, dt, :],
                        bass.AP(tensor=src.tensor, offset=src.offset,
                                ap=[[0, 64], [1, NQ]]),
                    )

            # ---------------- Phase 3: residual + fc_o ----------------
            # OPre = Vq + r*Oh + (1-p_q)*meanV, computed in place over oht_sb
            opre_sb = oht_sb
            for dt in range(NDT):
                nc.vector.tensor_mul(
                    opre_sb[:, dt, :], oht_sb[:, dt, :], rb_sb[:, dt, :]
                )
                nc.vector.tensor_add(
                    opre_sb[:, dt, :], opre_sb[:, dt, :], vtq_sb[:, dt, :]
                )
                nc.vector.scalar_tensor_tensor(
                    opre_sb[:, dt, :], ompq_bc, bias_sb["mv"][:, dt:dt + 1],
                    opre_sb[:, dt, :], OP.mult, OP.add,
                )
            for dt in range(NDT):
                ps = psA.tile([128, NQ], F32, tag="proj")
                for kt in range(NDT):
                    nc.tensor.matmul(
                        ps,
                        (w_sb["o"][:, kt, dt * 128:(dt + 1) * 128]),
                        (opre_sb[:, kt, :]),
                        start=(kt == 0), stop=(kt == NDT - 1),
                    )
                relu_sb = outp.tile([128, NQ], F32, tag="relu")
                nc.scalar.activation(
                    relu_sb, ps, AF.Relu, bias=bias_sb["o"][:, dt:dt + 1]
                )
                of_sb = outp.tile([128, NQ], F32, tag="of")
                nc.vector.tensor_add(of_sb, relu_sb, opre_sb[:, dt, :])
                nc.sync.dma_start(t_out[dt * 128:(dt + 1) * 128, :], of_sb)

    nc.compile()
    return nc


def make_in_maps(inputs):
    """Host-side prep: returns the per-core input dicts."""
    Y = np.asarray(inputs["Y_lift"], np.float32)
    X = np.asarray(inputs["X_pairs"], np.float32)
    pres = np.asarray(inputs["presence"], np.float32)
    Wq = np.asarray(inputs["Wq"], np.float32)
    Wk = np.asarray(inputs["Wk"], np.float32)
    Wv = np.asarray(inputs["Wv"], np.float32)
    Wo = np.asarray(inputs["Wo"], np.float32)
    bq = np.asarray(inputs["bq"], np.float32)
    bk = np.asarray(inputs["bk"], np.float32)
    bv = np.asarray(inputs["bv"], np.float32)
    bo = np.asarray(inputs["bo"], np.float32)
    b2 = np.asarray(inputs["b2"], np.float32)

    inv_sqrt = np.float32(1.0 / np.sqrt(D))
    WqT = np.ascontiguousarray(Wq.T * inv_sqrt)
    WkT = np.ascontiguousarray(Wk.T)
    WvT = np.ascontiguousarray(Wv.T)
    WoT = np.ascontiguousarray(Wo.T)

    Yt = np.ascontiguousarray(Y.transpose(0, 2, 1))            # (B, D, N)
    XT = np.ascontiguousarray(X.transpose(0, 3, 2, 1))          # (B, 3, k, q)
    V_full = Y @ Wv.T + bv                                      # (B, N, D) host
    meanV = V_full.mean(axis=1).astype(np.float32)              # (B, D)
    ident = np.eye(128, dtype=BF16NP)

    # fold 1/sqrt(D) scaling into bq too (Q^T evac bias rides the scaled path)
    bq_s = (bq * inv_sqrt).astype(np.float32)

    in_maps = []
    for c in range(8):
        b, qh = c // 2, c % 2
        qsl = slice(qh * NQ, (qh + 1) * NQ)
        pkb = (BIGNEG * (1.0 - pres[b])).astype(np.float32)     # (N,)
        expb = (pkb[None, :] + b2[:, None]).astype(np.float32)  # (H, N)
        in_maps.append({
            "ytq": np.ascontiguousarray(Yt[b][:, qsl]),
            "yt": Yt[b],
            "wqt": WqT, "wkt": WkT, "wvt": WvT, "wot": WoT,
            "xt": np.ascontiguousarray(XT[b][:, :, qsl]).astype(BF16NP),
            "maskq": (BIGNEG * (1.0 - pres[b, qsl])).astype(BF16NP).reshape(1, NQ),
            "expb": expb,
            "ompq": (1.0 - pres[b, qsl]).astype(np.float32).reshape(1, NQ),
            "bq": bq_s, "bk": bk, "bv": bv, "bo": bo,
            "bvrow": bv.reshape(1, D),
            "ones": np.ones((1, 128), np.float32),
            "mv": meanV[b],
            "ident": ident,
        })
    return in_maps


def assemble_output(results):
    out = np.empty((B, N, D), np.float32)
    for c in range(8):
        b, qh = c // 2, c % 2
        out[b, qh * NQ:(qh + 1) * NQ, :] = results[c]["out_t"].T
    return out


def kernel(**inputs):
    nc = build_program(inputs["W1"], inputs["b1"], inputs["W2"], inputs["b2"])
    in_maps = make_in_maps(inputs)
    trace = bool(int(os.environ.get("KERNEL_TRACE", "0")))
    res = bass_utils.run_bass_kernel_spmd(
        nc, in_maps, core_ids=list(range(8)), trace=trace
    )
    kernel.last_result = res
    return assemble_output(res.results)


# revision 19
# speedup vs baseline: 2.1187x; 1.2440x over previous
"""Trainium2 Bass kernel for nn_EqvTransformer (dense_transformer).

Sharding: 8 cores = 4 batches x 2 query-halves. Each core computes the full
attention output for its (batch, 512-query slice) over all 1024 keys and all
8 heads; no cross-core communication (fc_o is row-local).

Layout: "transposed attention" - logits are built as l^T[k, q] tiles (keys on
partitions, queries free):
  - content logits via a 65-row contract: rows 0-63 are K^T/Q^T for the head,
    row 64 is (ones | -1e30*(1-p_q)) so the absent-query mask rides the same
    matmul for free,
  - the pairwise-MLP location logits are accumulated into the same PSUM tile
    by identity-stationary matmuls (PE adds tiles for free),
  - exp() evacuation applies the key-mask + b2[h] as a per-partition ACT bias,
  - the softmax denominator falls out of the A.V matmul via a ones-column
    appended to V.

Pairwise MLP (the arithmetic bottleneck) is restructured for DVE perf modes:
per hidden unit, normalize by the largest |W1| coefficient p (ratios <= 1 so
bf16 stays accurate):
    t1 = x_c1 * r1 + b'      (tensor_scalar, 4x mode)
    t2 = x_c2 * r2           (tensor_scalar, 4x mode)
    z  = x_p + t1; z += t2   (tensor_tensor, 2x mode)
    r  = max/min(z, 0) * (a*W2)  (tensor_scalar, 4x mode; sign of a folds in)
then PSUM += I.r on the PE. scalar_tensor_tensor (no DVE perf modes) is
avoided entirely in the hot loop.

Absent queries (p_q=0): reference yields uniform A over ALL keys, i.e.
Oh[q] = mean(V). The -1e30 row zeroes the column, the denominator is fixed
(s += 1-p_q), and (1-p_q)*mean(V) is added back before fc_o.
"""

import sys, os

sys.path.insert(0, "/opt/trn_rl_repo")

import numpy as np
import ml_dtypes

import concourse.bass as bass
import concourse.tile as tile
from concourse import bacc, mybir
from concourse import bass_utils

B, N, D, H = 4, 1024, 512, 8
HD = D // H          # 64
NQ = 512             # queries per core
NKC = N // 128       # 8 key chunks of 128
NDT = D // 128       # 4 dout tiles of 128
KHALF = 2            # key halves for the MLP tiling
KCH = NKC // KHALF   # 4 key chunks per half
BIGNEG = -1.0e30

F32 = mybir.dt.float32
F32R = mybir.dt.float32r
BF16 = mybir.dt.bfloat16
AF = mybir.ActivationFunctionType
OP = mybir.AluOpType
BF16NP = ml_dtypes.bfloat16

# Units (h, o) routed through PE/ACT (z assembled in PSUM by identity
# matmuls, relu-evacuated by the scalar engine) instead of pure DVE.
N_ROUTE_B = int(os.environ.get("KERNEL_NB", "0"))


def _unit_params(W1, b1, W2):
    """Per (h, o): pivot channel, ratios, scales for the normalized MLP."""
    units = []
    for h in range(H):
        for o in range(3):
            w = [float(W1[h, o, c]) for c in range(3)]
            p = int(np.argmax(np.abs(w)))
            a = w[p]
            if a == 0.0:
                a = 1e-30
            c1, c2 = [c for c in range(3) if c != p]
            units.append({
                "h": h, "o": o, "p": p, "c1": c1, "c2": c2,
                "r1": w[c1] / a, "r2": w[c2] / a, "bn": float(b1[h, o]) / a,
                "a": a, "w2": float(W2[h, o]),
            })
    return units


def build_program(W1, b1, W2, b2):
    """Build the SPMD program (same for all 8 cores; per-core data differs).

    W1 (H,3,3), b1 (H,3), W2 (H,3), b2 (H,) are baked into instruction
    immediates (the kernel is compiled per call, so the weights are known).
    """
    W1 = np.asarray(W1, np.float64)
    b1 = np.asarray(b1, np.float64)
    W2 = np.asarray(W2, np.float64)
    units = _unit_params(W1, b1, W2)
    # Route-B set: units with the largest |a*w2| get PE/ACT treatment first
    # (arbitrary but deterministic); tuned via N_ROUTE_B.
    route_b = set()
    if N_ROUTE_B:
        order = sorted(range(24), key=lambda i: -abs(units[i]["a"] * units[i]["w2"]))
        route_b = set(order[:N_ROUTE_B])

    nc = bacc.Bacc("TRN2", target_bir_lowering=False, debug=False, num_devices=8)

    dram = {}

    def din(name, shape, dtype=F32):
        dram[name] = nc.dram_tensor(name, shape, dtype, kind="ExternalInput").ap()
        return dram[name]

    t_yt = din("yt", [D, N], F32R)           # raw Y^T full (proj rhs)
    t_wqt = din("wqt", [D, D], F32R)         # Wq.T / sqrt(D)
    t_wkt = din("wkt", [D, D], F32R)
    t_wvt = din("wvt", [D, D], F32R)
    t_wot = din("wot", [D, D], F32R)
    t_xt = din("xt", [3, N, NQ], BF16)       # X_pairs channel planes, [c, k, q]
    t_maskq = din("maskq", [1, NQ], BF16)    # -BIG*(1-p_q) row (bf16)
    t_expb = din("expb", [H, N], BF16)       # -BIG*(1-p_k) + b2[h]
    t_ompq = din("ompq", [1, NQ])            # 1 - p_q
    t_ompqb = din("ompqb", [1, NQ], BF16)    # 1 - p_q (bf16, for broadcast)
    t_bq = din("bq", [D])
    t_bk = din("bk", [D])
    t_bv = din("bv", [D])
    t_bo = din("bo", [D])
    t_mv = din("mv", [D])                    # mean(V) - bv (host)
    t_ident = din("ident", [128, 128], BF16)  # identity (acc stationary)
    t_out = nc.dram_tensor("out_t", [D, NQ], F32, kind="ExternalOutput").ap()

    qsl = slice(0, NQ)  # query slice within yt is host-prepared (yt IS the
    # full batch Y^T; the q-slice columns are passed via a separate view)
    t_ytq = din("ytq", [D, NQ], F32R)        # raw Y^T q-slice (Q/Vq rhs)

    with tile.TileContext(nc) as tc:
        with (
            tc.tile_pool(name="const", bufs=1) as const,
            tc.tile_pool(name="work", bufs=2) as work,
            tc.tile_pool(name="mlpt", bufs=2) as mlpt,
            tc.tile_pool(name="mlpr", bufs=2) as mlpr,
            tc.tile_pool(name="av", bufs=2) as avp,
            tc.tile_pool(name="outp", bufs=1) as outp,
            tc.tile_pool(name="psA", bufs=2, space="PSUM") as psA,
            tc.tile_pool(name="psL", bufs=(2 if route_b else 4), space="PSUM") as psL,
            tc.tile_pool(name="psO", bufs=2, space="PSUM") as psO,
            tc.tile_pool(name="dram", bufs=1, space="DRAM") as dramp,
        ):
            # ---------------- Phase 0: resident loads ----------------
            yt_sb = const.tile([128, NDT, N], F32R)
            nc.sync.dma_start(yt_sb, t_yt.rearrange("(dt p) n -> p dt n", p=128))
            ytq_sb = const.tile([128, NDT, NQ], F32R)
            nc.sync.dma_start(ytq_sb, t_ytq.rearrange("(dt p) n -> p dt n", p=128))
            w_sb = {}
            for nm, t in (("q", t_wqt), ("k", t_wkt), ("v", t_wvt), ("o", t_wot)):
                w_sb[nm] = const.tile([128, NDT, D], F32R, tag=f"w{nm}", name=f"w{nm}_sb")
                nc.sync.dma_start(w_sb[nm], t.rearrange("(kt p) d -> p kt d", p=128))
            xt_sb = const.tile([128, NKC, 3, NQ], BF16)
            for c in range(3):
                nc.sync.dma_start(
                    xt_sb[:, :, c, :],
                    t_xt[c].rearrange("(kc p) q -> p kc q", p=128),
                )
            expb_sb = const.tile([128, H, NKC], BF16)
            nc.sync.dma_start(expb_sb, t_expb.rearrange("h (kc p) -> p h kc", p=128))
            ompq_sb = const.tile([1, NQ], F32)
            nc.sync.dma_start(ompq_sb, t_ompq)
            src = t_ompqb[0:1, :]
            ompq_bc = const.tile([128, NQ], BF16)
            nc.sync.dma_start(
                ompq_bc,
                bass.AP(tensor=src.tensor, offset=src.offset, ap=[[0, 128], [1, NQ]]),
            )
            bias_sb = {}
            for nm, t in (("q", t_bq), ("k", t_bk), ("v", t_bv), ("o", t_bo), ("mv", t_mv)):
                bias_sb[nm] = const.tile([128, NDT], F32, tag=f"b{nm}", name=f"b{nm}_sb")
                nc.sync.dma_start(bias_sb[nm], t.rearrange("(dt p) -> p dt", p=128))
            ident_sb = const.tile([128, 128], BF16)
            nc.sync.dma_start(ident_sb, t_ident)
            # Scaled identities for route-B units: diag(r1), diag(r2) for the
            # PSUM z assembly; |W2| folds into the ACT evac so accumulation
            # uses a shared +/-I stationary.
            diag_sb = {}
            negi_sb = None
            if route_b:
                negi_sb = const.tile([128, 128], BF16, tag="negi", name="negi")
                nc.vector.tensor_scalar(negi_sb, ident_sb, -1.0, 0.0, OP.mult, OP.add)
            for i in sorted(route_b):
                u = units[i]
                d1 = const.tile([128, 128], BF16, tag=f"d1_{i}", name=f"d1_{i}")
                d2 = const.tile([128, 128], BF16, tag=f"d2_{i}", name=f"d2_{i}")
                nc.vector.tensor_scalar(d1, ident_sb, float(u["r1"]), 0.0, OP.mult, OP.add)
                nc.vector.tensor_scalar(d2, ident_sb, float(u["r2"]), 0.0, OP.mult, OP.add)
                diag_sb[i] = (d1, d2)

            # ---------------- Phase 1: projections ----------------
            # qt_h: [65, NQ] bf16 per head (rows 0-63 Q^T/sqrt(D), row 64 maskq)
            # kt_h: [65, N]  bf16 per head (rows 0-63 K^T, row 64 ones)
            qt_h = [const.tile([65, NQ], BF16, tag=f"qt{h}", name=f"qt{h}") for h in range(H)]
            kt_h = [const.tile([65, N], BF16, tag=f"kt{h}", name=f"kt{h}") for h in range(H)]
            for h in range(H):
                nc.sync.dma_start(qt_h[h][64:65, :], t_maskq)
                nc.vector.memset(kt_h[h][64:65, :], 1.0)
            for dt in range(NDT):
                ps = psA.tile([128, NQ], F32, tag="proj")
                for kt in range(NDT):
                    nc.tensor.matmul(
                        ps,
                        (w_sb["q"][:, kt, dt * 128:(dt + 1) * 128]),
                        (ytq_sb[:, kt, :]),
                        start=(kt == 0), stop=(kt == NDT - 1),
                    )
                for hh in range(2):
                    nc.scalar.activation(
                        qt_h[2 * dt + hh][0:64, :], ps[64 * hh:64 * hh + 64, :],
                        AF.Identity, bias=bias_sb["q"][64 * hh:64 * hh + 64, dt:dt + 1],
                    )
                for th in range(2):
                    ps2 = psA.tile([128, NQ], F32, tag="proj")
                    for kt in range(NDT):
                        nc.tensor.matmul(
                            ps2,
                            (w_sb["k"][:, kt, dt * 128:(dt + 1) * 128]),
                            (yt_sb[:, kt, th * NQ:(th + 1) * NQ]),
                            start=(kt == 0), stop=(kt == NDT - 1),
                        )
                    for hh in range(2):
                        nc.scalar.activation(
                            kt_h[2 * dt + hh][0:64, th * NQ:(th + 1) * NQ],
                            ps2[64 * hh:64 * hh + 64, :],
                            AF.Identity,
                            bias=bias_sb["k"][64 * hh:64 * hh + 64, dt:dt + 1],
                        )
            # V natural [token-part, dout-free] (bf16, with ones column per head)
            v_sb = const.tile([128, NKC, H, HD + 1], BF16)
            nc.vector.memset(v_sb[:, :, :, HD:HD + 1], 1.0)
            for tt in range(NKC):
                ps = psA.tile([128, D], F32, tag="proj")
                for kt in range(NDT):
                    nc.tensor.matmul(
                        ps,
                        (yt_sb[:, kt, tt * 128:(tt + 1) * 128]),
                        (w_sb["v"][:, kt, :]),
                        start=(kt == 0), stop=(kt == NDT - 1),
                    )
                nc.scalar.activation(
                    v_sb[:, tt, :, 0:HD], ps.rearrange("p (h d) -> p h d", h=H),
                    AF.Identity,
                )
            # V^T for the query slice [dout-part, q-free] (residual + fc_o input)
            vtq_sb = const.tile([128, NDT, NQ], BF16)
            for dt in range(NDT):
                ps = psA.tile([128, NQ], F32, tag="proj")
                for kt in range(NDT):
                    nc.tensor.matmul(
                        ps,
                        (w_sb["v"][:, kt, dt * 128:(dt + 1) * 128]),
                        (ytq_sb[:, kt, :]),
                        start=(kt == 0), stop=(kt == NDT - 1),
                    )
                nc.scalar.activation(
                    vtq_sb[:, dt, :], ps, AF.Identity, bias=bias_sb["v"][:, dt:dt + 1]
                )

            # ---------------- Phase 2: attention ----------------
            oht_sb = const.tile([128, NDT, NQ], F32R)
            rb_dram = dramp.tile([H, NQ], F32)
            for h in range(H):
                po = psO.tile([HD + 1, NQ], F32, tag="po")
                for kh in range(KHALF):
                    # --- pairwise MLP for this (head, key-half) ---
                    r_tiles = []
                    for o in range(3):
                        u = units[3 * h + o]
                        i_u = 3 * h + o
                        xp = xt_sb[:, kh * KCH:(kh + 1) * KCH, u["p"], :]
                        x1 = xt_sb[:, kh * KCH:(kh + 1) * KCH, u["c1"], :]
                        x2 = xt_sb[:, kh * KCH:(kh + 1) * KCH, u["c2"], :]
                        if i_u not in route_b:
                            t1 = mlpt.tile([128, KCH, NQ], BF16, tag="t1")
                            t2 = mlpt.tile([128, KCH, NQ], BF16, tag="t2")
                            nc.vector.tensor_scalar(
                                t1, x1, u["r1"], u["bn"], OP.mult, OP.add
                            )
                            nc.vector.tensor_scalar(
                                t2, x2, u["r2"], 0.0, OP.mult, OP.add
                            )
                            nc.vector.tensor_add(t1, t1, xp)
                            nc.vector.tensor_add(t1, t1, t2)
                            r = mlpr.tile([128, KCH, NQ], BF16, tag=f"r{o}")
                            nc.vector.tensor_scalar(
                                r, t1, 0.0, u["a"] * u["w2"],
                                OP.max if u["a"] > 0 else OP.min, OP.mult,
                            )
                            r_tiles.append((r, ident_sb))
                        else:
                            # route B: zero-DVE. PE assembles z-hat in PSUM via
                            # scaled-diagonal stationaries; ACT relu-evacuates
                            # with scale=a (denormalization) and bias=b1.
                            d1, d2 = diag_sb[i_u]
                            r = mlpr.tile([128, KCH, NQ], BF16, tag=f"r{o}")
                            for j in range(KCH):
                                psz = psL.tile([128, NQ], F32, tag="z")
                                nc.tensor.matmul(psz, ident_sb, xp[:, j, :],
                                                 start=True, stop=False)
                                nc.tensor.matmul(psz, d1, x1[:, j, :],
                                                 start=False, stop=False)
                                nc.tensor.matmul(psz, d2, x2[:, j, :],
                                                 start=False, stop=True)
                                sc = float(u["a"] * abs(u["w2"]))
                                nc.scalar.activation(
                                    r[:, j, :], psz, AF.Relu,
                                    scale=sc, bias=float(sc * u["bn"]),
                                )
                            r_tiles.append(
                                (r, ident_sb if u["w2"] > 0 else negi_sb))
                    # --- content logits + MLP accumulation + exp + A.V ---
                    ps_tiles = []
                    for j in range(KCH):
                        kc = kh * KCH + j
                        ps = psL.tile([128, NQ], F32, tag="l")
                        nc.tensor.matmul(
                            ps,
                            (kt_h[h][:, kc * 128:(kc + 1) * 128]),
                            (qt_h[h]),
                            start=True, stop=False,
                        )
                        ps_tiles.append(ps)
                    for j in range(KCH):
                        ps = ps_tiles[j]
                        for o in range(3):
                            r, stat = r_tiles[o]
                            nc.tensor.matmul(
                                ps, stat, r[:, j, :],
                                start=False, stop=(o == 2),
                            )
                    for j in range(KCH):
                        kc = kh * KCH + j
                        a = avp.tile([128, NQ], BF16, tag="a")
                        nc.scalar.activation(
                            a, ps_tiles[j], AF.Exp, bias=expb_sb[:, h, kc:kc + 1]
                        )
                        nc.tensor.matmul(
                            po, v_sb[:, kc, h, :], a,
                            start=(kc == 0), stop=(kc == NKC - 1),
                        )
                # denominator row (+ (1-p_q) fix, approx reciprocal) + Oh^T evac
                s_sb = work.tile([1, NQ], F32, tag="s", bufs=2)
                nc.vector.tensor_add(s_sb, po[HD:HD + 1, :], ompq_sb)
                rt = work.tile([1, NQ], F32, tag="rden", bufs=2)
                nc.vector.reciprocal_approx_fast(rt, s_sb)
                nc.sync.dma_start(rb_dram[h:h + 1, :], rt)
                nc.scalar.activation(
                    oht_sb[64 * (h % 2):64 * (h % 2) + 64, h // 2, :], po[0:HD, :],
                    AF.Identity,
                )

            rb_sb = const.tile([128, NDT, NQ], F32)
            for dt in range(NDT):
                for hh in range(2):
                    src = rb_dram[2 * dt + hh:2 * dt + hh + 1, :]
                    nc.sync.dma_start(
                        rb_sb[64 * hh:64 * hh + 64, dt, :],
                        bass.AP(tensor=src.tensor, offset=src.offset,
                                ap=[[0, 64], [1, NQ]]),
                    )

            # ---------------- Phase 3: residual + fc_o ----------------
            # OPre = Vq + r*Oh + (1-p_q)*meanV, computed in place over oht_sb
            opre_sb = oht_sb
            for dt in range(NDT):
                nc.vector.tensor_mul(
                    opre_sb[:, dt, :], oht_sb[:, dt, :], rb_sb[:, dt, :]
                )
                nc.vector.tensor_add(
                    opre_sb[:, dt, :], opre_sb[:, dt, :], vtq_sb[:, dt, :]
                )
                nc.vector.scalar_tensor_tensor(
                    opre_sb[:, dt, :], ompq_bc, bias_sb["mv"][:, dt:dt + 1],
                    opre_sb[:, dt, :], OP.mult, OP.add,
                )
            for dt in range(NDT):
                ps = psA.tile([128, NQ], F32, tag="proj")
                for kt in range(NDT):
                    nc.tensor.matmul(
                        ps,
                        (w_sb["o"][:, kt, dt * 128:(dt + 1) * 128]),
                        (opre_sb[:, kt, :]),
                        start=(kt == 0), stop=(kt == NDT - 1),
                    )
                relu_sb = outp.tile([128, NQ], F32, tag="relu", bufs=1)
                nc.scalar.activation(
                    relu_sb, ps, AF.Relu, bias=bias_sb["o"][:, dt:dt + 1]
                )
                nc.vector.tensor_add(relu_sb, relu_sb, opre_sb[:, dt, :])
                nc.sync.dma_start(t_out[dt * 128:(dt + 1) * 128, :], relu_sb)

    nc.compile()
    return nc


def make_in_maps(inputs):
    """Host-side prep: returns the per-core input dicts."""
    Y = np.asarray(inputs["Y_lift"], np.float32)
    X = np.asarray(inputs["X_pairs"], np.float32)
    pres = np.asarray(inputs["presence"], np.float32)
    Wq = np.asarray(inputs["Wq"], np.float32)
    Wk = np.asarray(inputs["Wk"], np.float32)
    Wv = np.asarray(inputs["Wv"], np.float32)
    Wo = np.asarray(inputs["Wo"], np.float32)
    bq = np.asarray(inputs["bq"], np.float32)
    bk = np.asarray(inputs["bk"], np.float32)
    bv = np.asarray(inputs["bv"], np.float32)
    bo = np.asarray(inputs["bo"], np.float32)
    b2 = np.asarray(inputs["b2"], np.float32)

    inv_sqrt = np.float32(1.0 / np.sqrt(D))
    WqT = np.ascontiguousarray(Wq.T * inv_sqrt)
    WkT = np.ascontiguousarray(Wk.T)
    WvT = np.ascontiguousarray(Wv.T)
    WoT = np.ascontiguousarray(Wo.T)

    Yt = np.ascontiguousarray(Y.transpose(0, 2, 1))            # (B, D, N)
    XT = np.ascontiguousarray(X.transpose(0, 3, 2, 1))          # (B, 3, k, q)
    V_full = Y @ Wv.T + bv                                      # (B, N, D) host
    meanV = V_full.mean(axis=1).astype(np.float32)              # (B, D)
    ident = np.eye(128, dtype=BF16NP)

    # fold 1/sqrt(D) scaling into bq too (Q^T evac bias rides the scaled path)
    bq_s = (bq * inv_sqrt).astype(np.float32)

    in_maps = []
    for c in range(8):
        b, qh = c // 2, c % 2
        qsl = slice(qh * NQ, (qh + 1) * NQ)
        pkb = (BIGNEG * (1.0 - pres[b])).astype(np.float32)     # (N,)
        expb = (pkb[None, :] + b2[:, None]).astype(BF16NP)      # (H, N)
        in_maps.append({
            "ytq": np.ascontiguousarray(Yt[b][:, qsl]),
            "yt": Yt[b],
            "wqt": WqT, "wkt": WkT, "wvt": WvT, "wot": WoT,
            "xt": np.ascontiguousarray(XT[b][:, :, qsl]).astype(BF16NP),
            "maskq": (BIGNEG * (1.0 - pres[b, qsl])).astype(BF16NP).reshape(1, NQ),
            "expb": expb,
            "ompq": (1.0 - pres[b, qsl]).astype(np.float32).reshape(1, NQ),
            "ompqb": (1.0 - pres[b, qsl]).astype(BF16NP).reshape(1, NQ),
            "bq": bq_s, "bk": bk, "bv": 2.0 * bv, "bo": bo,
            "mv": meanV[b] - bv,
            "ident": ident,
        })
    return in_maps


def assemble_output(results):
    out = np.empty((B, N, D), np.float32)
    for c in range(8):
        b, qh = c // 2, c % 2
        out[b, qh * NQ:(qh + 1) * NQ, :] = results[c]["out_t"].T
    return out


def kernel(**inputs):
    nc = build_program(inputs["W1"], inputs["b1"], inputs["W2"], inputs["b2"])
    in_maps = make_in_maps(inputs)
    trace = bool(int(os.environ.get("KERNEL_TRACE", "0")))
    res = bass_utils.run_bass_kernel_spmd(
        nc, in_maps, core_ids=list(range(8)), trace=trace
    )
    kernel.last_result = res
    return assemble_output(res.results)
